# revision 27
# baseline (speedup 1.0000x reference)
"""Multi-head attention (B=2, S=2048, D=1024, H=16) on 8 TRN2 NeuronCores.

Sharding: tensor-parallel over heads x data-parallel over batch.
Core c handles batch b = c // 4 and head group g = c % 4 (4 heads each).
Each core computes its 4 heads' q/k/v projections, attention, and the
partial output projection against its slice of Wo; the host sums the 4
partials per batch element.

Schedule (v2): the kernel is ACT(exp)-bound in the attention phase (128
exp tiles of [128,1024] at ~1us each), so everything is arranged to
start the exp stream as early as possible and keep it dense:
  - K then Q (pair 0 only) project first -> scores (pair0, chunk0)
    start ~15us in, with exp(k2) issued right behind each score pair.
  - The two per-pair score matmuls contract over Dh=64 on partition
    ranges 0:64/64:128 -> tile_position row groups (0,0)/(64,0) run
    CONCURRENTLY on the PE array (~2x scores).
  - V is projected directly into natural [seq, feat] layout (stationary
    = xT seq-tile, moving = Wv) -- no PE transposes -- as side-work
    inside attention chunk 0, just-in-time for the AV matmuls which lag
    one chunk behind the exp stream (et pool is deep: 20 tiles).
  - Remaining projections (pair 1) and the output projection are
    interleaved as fine-grained side-work between attention units, so
    the output DMA is spread across the kernel instead of tailing it.

fp16 streaming everywhere; accumulation in f32 PSUM. The AV matmul's
stationary carries 64 columns of ones so softmax row-sums come out of
the same matmul (partition-broadcast is illegal on DVE; replicating in
the matmul is free). Normalization (reciprocal * mult) on DVE; psum
evacuations on DVE/Pool; ACT runs exp only.
"""

import os

import numpy as np

# bisection switches (dev only; default full-featured)
KV = set(os.environ.get("KV", "").split(","))

B, S, D, H, DH = 2, 2048, 1024, 16, 64
NCORES = 8
GROUPS = 4  # head groups; 4 heads = 256 features per core
M = 256  # head features per core
SCALE = 0.125  # 1/sqrt(64)

_compiled = None


def _build_module():
    import concourse.mybir as mybir
    import concourse.tile as tile
    from concourse import bacc

    f32 = mybir.dt.float32
    fp16 = mybir.dt.float16
    nc = bacc.Bacc("TRN2", target_bir_lowering=False, debug=False,
                   num_devices=NCORES)
    xT = nc.dram_tensor("xT", [D, S], fp16, kind="ExternalInput").ap()
    wq = nc.dram_tensor("wq", [D, M], fp16, kind="ExternalInput").ap()
    wk = nc.dram_tensor("wk", [D, M], fp16, kind="ExternalInput").ap()
    wv = nc.dram_tensor("wv", [D, M], fp16, kind="ExternalInput").ap()
    wo = nc.dram_tensor("wo", [M, D], fp16, kind="ExternalInput").ap()
    out = nc.dram_tensor("out", [S, D], f32, kind="ExternalOutput").ap()

    with tile.TileContext(nc) as tc:
        _kernel_body(tc, out, xT, wq, wk, wv, wo)
    nc.compile()
    return nc


def _kernel_body(tc, out, xT, wq, wk, wv, wo):
    from contextlib import ExitStack

    import concourse.mybir as mybir
    from concourse.masks import make_identity

    nc = tc.nc
    f32 = mybir.dt.float32
    fp16 = mybir.dt.float16
    AF = mybir.ActivationFunctionType

    P = 128
    NKT = D // P   # 8 k-tiles in the projection contraction
    NPT = M // P   # 2 partition-tiles of head features (head pairs)
    SKT = S // P   # 16 key tiles
    QC = 512       # q chunk (psum bank width in f32)
    NQC = S // QC  # 4
    HPC = 4        # heads per core

    with ExitStack() as ctx:
        const = ctx.enter_context(tc.tile_pool(name="const", bufs=1))
        big = ctx.enter_context(tc.tile_pool(name="big", bufs=1))
        work = ctx.enter_context(tc.tile_pool(name="work", bufs=2))
        exp_pool = ctx.enter_context(
            tc.tile_pool(name="exp", bufs=20 if "lag" in KV else 6))
        small = ctx.enter_context(tc.tile_pool(name="small", bufs=2))
        # PSUM budget (8 banks): psA 2 + psS 2x2 + psO 2x1 = 8
        psum_big = ctx.enter_context(tc.tile_pool(name="psA", bufs=2, space="PSUM"))
        psum_s = ctx.enter_context(tc.tile_pool(name="psS", bufs=2, space="PSUM"))
        psum_o = ctx.enter_context(tc.tile_pool(name="psO", bufs=1, space="PSUM"))

        ident_f = const.tile([P, P], f32)
        make_identity(nc, ident_f)
        ident = const.tile([P, P], fp16, tag="ident_h")
        nc.vector.tensor_copy(ident[:], ident_f[:])

        QT = big.tile([P, NPT, S], fp16, tag="QT")
        KT = big.tile([P, NPT, S], fp16, tag="KT")
        OT = big.tile([P, NPT, S], fp16, tag="OT")
        VA = big.tile([P, HPC, SKT, P], fp16, tag="VA")
        wo_sb = big.tile([P, NPT, D], fp16, tag="wo")

        # --- input DMA ---
        # weights on the scalar HWDGE queue in consumption order, bulk xT
        # on the sync HWDGE queue; wo last (not needed until outproj).
        w_sb = {}
        for name, w in (("k", wk), ("q", wq), ("v", wv)):
            t = big.tile([P, NKT, M], fp16, tag=f"w{name}")
            nc.scalar.dma_start(t[:], w.rearrange("(kt p) m -> p kt m", p=P))
            w_sb[name] = t

        xT_sb = big.tile([P, NKT, S], fp16, tag="xT")
        xT_r = xT.rearrange("(kt p) s -> p kt s", p=P)
        for c in range(NQC):
            for kh in range(4 if c == 0 else 2):
                n = NKT // (4 if c == 0 else 2)
                nc.sync.dma_start(
                    xT_sb[:, kh * n:(kh + 1) * n, c * QC:(c + 1) * QC],
                    xT_r[:, kh * n:(kh + 1) * n, c * QC:(c + 1) * QC])
        nc.sync.dma_start(wo_sb[:], wo.rearrange("(pt p) n -> p pt n", p=P))

        # ones block of VA for softmax row-sums
        ones_f = const.tile([P, 64], fp16, tag="ones")
        nc.gpsimd.memset(ones_f[:], 1.0)
        for h in range(HPC):
            for st in range(SKT):
                nc.gpsimd.tensor_copy(VA[:, h, st, 64:128], ones_f[:])

        # warm the PE clock (HAM) during the input DMA head
        for _ in range(24):
            warm_ps = psum_big.tile([P, QC], f32, tag="ps_big")
            nc.tensor.matmul(warm_ps[:, 0:P], ident[:], ident[:],
                             start=True, stop=True)

        def proj_group(name, pt, c):
            """One projection psum fill (8 accumulating matmuls + evac),
            split into 8 single-matmul side steps sharing lazy state."""
            state = {}
            dst = {"q": QT, "k": KT}[name]
            cs = slice(c * QC, (c + 1) * QC)

            def step(kt, state=state, cs=cs, dst=dst, name=name, pt=pt):
                if kt == 0:
                    state["ps"] = psum_big.tile([P, QC], f32, tag="ps_big",
                                                name="ps_proj")
                ps = state["ps"]
                nc.tensor.matmul(
                    ps[:], w_sb[name][:, kt, pt * P:(pt + 1) * P],
                    xT_sb[:, kt, cs],
                    start=(kt == 0), stop=(kt == NKT - 1))
                if kt == NKT - 1:
                    nc.vector.tensor_copy(dst[:, pt, cs], ps[:])

            return [lambda kt=kt: step(kt) for kt in range(NKT)]

        if "novdirect" in KV:
            f32r = mybir.dt.float32r
            ident_r = const.tile([P, P], f32r, tag="ident_r")
            nc.vector.tensor_copy(ident_r[:], ident_f[:])
            VT = big.tile([P, NPT, S], f32r, tag="VT")

            def v_group(st):
                # baseline path: transposed V proj (once, upfront via novjit)
                def thunk(st=st):
                    if st == 0:
                        for pt in range(NPT):
                            for c in range(NQC):
                                ps = psum_big.tile([P, QC], f32, tag="ps_big")
                                cs = slice(c * QC, (c + 1) * QC)
                                for kt in range(NKT):
                                    nc.tensor.matmul(
                                        ps[:],
                                        w_sb["v"][:, kt, pt * P:(pt + 1) * P],
                                        xT_sb[:, kt, cs],
                                        start=(kt == 0), stop=(kt == NKT - 1))
                                nc.vector.tensor_copy(VT[:, pt, cs], ps[:])
                    for pt in range(NPT):
                        pst = psum_big.tile([P, P], f32r, tag="ps_big")
                        nc.tensor.transpose(
                            pst[:], VT[:, pt, st * P:(st + 1) * P], ident_r)
                        nc.vector.tensor_copy(VA[:, 2 * pt, st, 0:64],
                                              pst[:, 0:64])
                        nc.vector.tensor_copy(VA[:, 2 * pt + 1, st, 0:64],
                                              pst[:, 64:128])
                return thunk
        else:
            def v_group(st):
                """Project V for seq-tile st directly into natural layout and
                scatter per-head slices (+CAST) into VA. One side-thunk."""
                def thunk(st=st):
                    ps = psum_big.tile([P, QC], f32, tag="ps_big", name="ps_v")
                    ss = slice(st * P, (st + 1) * P)
                    for kt in range(NKT):
                        nc.tensor.matmul(
                            ps[:, 0:M], xT_sb[:, kt, ss], w_sb["v"][:, kt, :],
                            start=(kt == 0), stop=(kt == NKT - 1))
                    for h in range(HPC):
                        nc.vector.tensor_copy(VA[:, h, st, 0:64],
                                              ps[:, h * 64:(h + 1) * 64])
                return thunk

        def outproj_piece(qt, nch):
            """Two single-matmul steps; the second evacuates + DMAs."""
            state = {}

            def step(pt, qt=qt, nch=nch, state=state):
                if pt == 0:
                    state["ps"] = psum_big.tile([P, QC], f32, tag="ps_big",
                                                name="ps_op")
                ps = state["ps"]
                nc.tensor.matmul(
                    ps[:], OT[:, pt, qt * P:(qt + 1) * P],
                    wo_sb[:, pt, nch * QC:(nch + 1) * QC],
                    start=(pt == 0), stop=(pt == NPT - 1))
                if pt == NPT - 1:
                    ot = work.tile([P, QC], f32, tag="outstage")
                    nc.vector.tensor_copy(ot[:], ps[:])
                    nc.sync.dma_start(
                        out[qt * P:(qt + 1) * P, nch * QC:(nch + 1) * QC],
                        ot[:])

            return [lambda pt=pt: step(pt) for pt in range(NPT)]

        # --- K, Q (pair 0, first q-chunk, first key window) so scores can
        # start as soon as the first xT chunk lands; sc(0,k2) only reads
        # KT key-tile k2, so the remaining K chunks project as early
        # side-work just ahead of their first consumer. ---
        for step in proj_group("k", 0, 0):
            step()
        for step in proj_group("q", 0, 0):
            step()

        if "noside" in KV:
            # bisect variant: all projections upfront, no interleave
            for c in range(1, NQC):
                for step in proj_group("q", 0, c):
                    step()
            for name in ("k", "q"):
                for c in range(NQC):
                    for step in proj_group(name, 1, c):
                        step()

        # --- attention pipeline ---
        # chunks in pair-major order; AV lags the exp stream by one chunk
        # (et pool is deep), side-work interleaved per k2 unit.
        chunks = [(p, c) for p in range(NPT) for c in range(NQC)]

        # side-work lists per chunk (index 9 = tail), scripted so every
        # projection lands >= one chunk before its first consumer; the
        # light outproj chunks (4-7) absorb pair-1 Q so the exp-critical
        # early chunks stay close to the ACT cadence.
        side = {i: [] for i in range(10)}
        if "noside" not in KV:
            side[0] += proj_group("k", 0, 1) + proj_group("k", 0, 2) \
                + proj_group("k", 0, 3) + proj_group("q", 0, 1)
            side[1] += proj_group("q", 0, 2) + proj_group("k", 1, 0)
            side[2] += proj_group("q", 0, 3) + proj_group("k", 1, 1)
            side[3] += proj_group("k", 1, 2) + proj_group("k", 1, 3) \
                + proj_group("q", 1, 0)
            side[4] += proj_group("q", 1, 1)
            side[5] += proj_group("q", 1, 2)
            side[6] += proj_group("q", 1, 3)
        # outproj thunks for window c are appended to side[...] by norm()

        vq = [v_group(st) for st in range(SKT)]
        if "novjit" in KV:
            for th in vq:
                th()
            vq = [(lambda: None) for _ in range(SKT)]
        ets = {}     # global unit -> et tile
        pos = {}     # chunk_idx -> (poA, poB)

        def sc_unit(u):
            i, k2 = divmod(u, SKT)
            p, c = chunks[i]
            cs = slice(c * QC, (c + 1) * QC)
            ks = slice(k2 * P, (k2 + 1) * P)
            pss = psum_s.tile([P, 2, QC], f32, tag="ps_s")
            for j in range(2):
                nc.tensor.matmul(pss[:, j, :], KT[64 * j:64 * (j + 1), p, ks],
                                 QT[64 * j:64 * (j + 1), p, cs],
                                 start=True, stop=True)
            et = exp_pool.tile([P, 2, QC], fp16, tag="exp")
            nc.scalar.activation(et[:], pss[:], AF.Exp, scale=SCALE)
            ets[u] = et

        def av_unit(u):
            i, k2 = divmod(u, SKT)
            p, c = chunks[i]
            et = ets.pop(u)
            if k2 == 0:
                poA = psum_o.tile([P, QC], f32, tag="ps_oA")
                poB = psum_o.tile([P, QC], f32, tag="ps_oB")
                pos[i] = (poA, poB)
            poA, poB = pos[i]
            for j, po in ((0, poA), (1, poB)):
                nc.tensor.matmul(po[:], VA[:, 2 * p + j, k2, :], et[:, j, :],
                                 start=(k2 == 0), stop=(k2 == SKT - 1))
            if k2 == SKT - 1:
                norm(i)

        def norm(i):
            p, c = chunks[i]
            cs = slice(c * QC, (c + 1) * QC)
            for r0, po in ((0, pos[i][0]), (64, pos[i][1])):
                pc = small.tile([P, QC], f32, tag="po_sb")
                nc.vector.tensor_copy(pc[:], po[:])
                sm = small.tile([64, QC], f32, tag="sums")
                nc.vector.tensor_copy(sm[:], pc[64:128, :])
                rb = small.tile([64, QC], f32, tag="recip")
                nc.vector.reciprocal_approx_fast(rb[:], sm[:])
                nc.vector.tensor_tensor(
                    OT[r0:r0 + 64, p, cs], pc[0:64, :], rb[:],
                    mybir.AluOpType.mult)
            del pos[i]
            if p == 1:
                # OT complete for this q window: queue its outproj
                tgt = 9 if ("noopi" in KV or i == 7) else i + 1
                for qt in range(c * (QC // P), (c + 1) * (QC // P)):
                    for nch in range(2):
                        side[tgt].extend(outproj_piece(qt, nch))

        # --- global unit pipeline: av (lagging LAG units, emitted first so
        # its wait absorbs the exp latency), then side-work, then sc ---
        LAG = 2
        NU = 8 * SKT
        for u in range(NU):
            i, k2 = divmod(u, SKT)
            if u >= LAG:
                av_unit(u - LAG)
            if i == 0:
                vq[k2]()
            sq = side[i]
            for _ in range(min(2, len(sq))):
                sq.pop(0)()
            if k2 == SKT - 1:
                while sq:
                    sq.pop(0)()
            sc_unit(u)
        for u in range(NU - LAG, NU):
            av_unit(u)
        # keep the PE clock hot through the final norm so the tail outproj
        # runs warm
        for _ in range(10):
            wp = psum_big.tile([P, QC], f32, tag="ps_big", name="tailwarm")
            nc.tensor.matmul(wp[:, 0:P], ident[:], ident[:],
                             start=True, stop=True)
        for th in side[8] + side[9]:
            th()
        assert not ets and not pos


def _in_maps(x, Wq, Wk, Wv, Wo):
    x = np.asarray(x, dtype=np.float32)
    Wq = np.asarray(Wq, dtype=np.float32)
    Wk = np.asarray(Wk, dtype=np.float32)
    Wv = np.asarray(Wv, dtype=np.float32)
    Wo = np.asarray(Wo, dtype=np.float32)
    xT = [np.ascontiguousarray(x[b].T).astype(np.float16) for b in range(B)]
    maps = []
    for c in range(NCORES):
        b, g = c // GROUPS, c % GROUPS
        rows = slice(g * M, (g + 1) * M)
        maps.append({
            "xT": xT[b],
            "wq": np.ascontiguousarray(Wq[rows, :].T).astype(np.float16),
            "wk": np.ascontiguousarray(Wk[rows, :].T).astype(np.float16),
            "wv": np.ascontiguousarray(Wv[rows, :].T).astype(np.float16),
            "wo": np.ascontiguousarray(Wo[:, rows].T).astype(np.float16),
        })
    return maps


def kernel(x, Wq, Wk, Wv, Wo, _trace=False):
    global _compiled
    if _compiled is None:
        _compiled = _build_module()
    from concourse.bass_utils import run_bass_kernel_spmd

    res = run_bass_kernel_spmd(
        _compiled, _in_maps(x, Wq, Wk, Wv, Wo),
        core_ids=list(range(NCORES)), trace=_trace,
    )
    outs = [r["out"] for r in res.results]
    y = np.empty((B, S, D), np.float32)
    for b in range(B):
        y[b] = outs[4 * b] + outs[4 * b + 1] + outs[4 * b + 2] + outs[4 * b + 3]
    if _trace:
        kernel.last_results = res
    return y


# revision 29
# speedup vs baseline: 1.0142x; 1.0142x over previous
"""Multi-head attention (B=2, S=2048, D=1024, H=16) on 8 TRN2 NeuronCores.

Sharding: tensor-parallel over heads x data-parallel over batch.
Core c handles batch b = c // 4 and head group g = c % 4 (4 heads each).
Each core computes its 4 heads' q/k/v projections, attention, and the
partial output projection against its slice of Wo; the host sums the 4
partials per batch element.

Schedule: the attention phase is paced by the ACT engine's exp stream
(128 exp tiles of [128,1024] at ~1.1us each), with the PE near-saturated
around it, so the kernel is organized as a single software-pipelined
stream of 128 "units" (8 chunks of (head-pair, q-window) x 16 key
tiles):
  - Only K and Q for (pair 0, q-window 0) project up front; sc(0,k2)
    reads just key-tile k2, so scoring starts as soon as the first xT
    DMA chunk lands. All remaining projections are split into
    single-matmul side-steps popped between units, each scheduled at
    least one chunk before its first consumer.
  - Unit u: AV for unit u-2 (its exp long done - the wait absorbs the
    exp->pss round-trip), side-steps, then scores for unit u; the two
    per-pair score matmuls contract Dh=64 on partition ranges
    0:64/64:128 so their tile_position row groups (0,0)/(64,0) run
    concurrently on the PE array.
  - V is projected directly into natural [seq, feat] layout (stationary
    = xT seq-tile, moving = Wv; no PE transposes), just-in-time per key
    tile inside chunk 0.
  - The output projection is split into single-matmul side-steps that
    follow each q-window's completion, spreading the output DMA across
    the kernel; ident matmuls keep the PE clock (HAM) warm through the
    final normalization so the tail outproj runs at full clock.

fp16 streaming everywhere; accumulation in f32 PSUM. The AV matmul's
stationary carries 64 columns of ones so softmax row-sums come out of
the same matmul (partition-broadcast is illegal on DVE; replicating in
the matmul is free). Normalization (reciprocal * mult) on DVE via two
aligned 64-row evacuations; note reciprocal_approx_fast requires a
base-partition-0 input on real hardware (a partition-shifted input
returns garbage even though CoreSim accepts it). ACT runs exp only.
"""

import os

import numpy as np

# bisection switches (dev only; default full-featured)
KV = set(os.environ.get("KV", "").split(","))

B, S, D, H, DH = 2, 2048, 1024, 16, 64
NCORES = 8
GROUPS = 4  # head groups; 4 heads = 256 features per core
M = 256  # head features per core
SCALE = 0.125  # 1/sqrt(64)

_compiled = None


def _build_module():
    import concourse.mybir as mybir
    import concourse.tile as tile
    from concourse import bacc

    f32 = mybir.dt.float32
    fp16 = mybir.dt.float16
    nc = bacc.Bacc("TRN2", target_bir_lowering=False, debug=False,
                   num_devices=NCORES)
    xT = nc.dram_tensor("xT", [D, S], fp16, kind="ExternalInput").ap()
    wq = nc.dram_tensor("wq", [D, M], fp16, kind="ExternalInput").ap()
    wk = nc.dram_tensor("wk", [D, M], fp16, kind="ExternalInput").ap()
    wv = nc.dram_tensor("wv", [D, M], fp16, kind="ExternalInput").ap()
    wo = nc.dram_tensor("wo", [M, D], fp16, kind="ExternalInput").ap()
    out = nc.dram_tensor("out", [S, D], f32, kind="ExternalOutput").ap()

    with tile.TileContext(nc) as tc:
        _kernel_body(tc, out, xT, wq, wk, wv, wo)
    nc.compile()
    return nc


def _kernel_body(tc, out, xT, wq, wk, wv, wo):
    from contextlib import ExitStack

    import concourse.mybir as mybir
    from concourse.masks import make_identity

    nc = tc.nc
    f32 = mybir.dt.float32
    fp16 = mybir.dt.float16
    AF = mybir.ActivationFunctionType

    P = 128
    NKT = D // P   # 8 k-tiles in the projection contraction
    NPT = M // P   # 2 partition-tiles of head features (head pairs)
    SKT = S // P   # 16 key tiles
    QC = 512       # q chunk (psum bank width in f32)
    NQC = S // QC  # 4
    HPC = 4        # heads per core

    with ExitStack() as ctx:
        const = ctx.enter_context(tc.tile_pool(name="const", bufs=1))
        big = ctx.enter_context(tc.tile_pool(name="big", bufs=1))
        work = ctx.enter_context(tc.tile_pool(name="work", bufs=2))
        exp_pool = ctx.enter_context(
            tc.tile_pool(name="exp", bufs=20 if "lag" in KV else 6))
        small = ctx.enter_context(tc.tile_pool(name="small", bufs=2))
        # PSUM budget (8 banks): psA 2 + psS 2x2 + psO 2x1 = 8
        psum_big = ctx.enter_context(tc.tile_pool(name="psA", bufs=2, space="PSUM"))
        psum_s = ctx.enter_context(tc.tile_pool(name="psS", bufs=2, space="PSUM"))
        psum_o = ctx.enter_context(tc.tile_pool(name="psO", bufs=1, space="PSUM"))

        ident_f = const.tile([P, P], f32)
        make_identity(nc, ident_f)
        ident = const.tile([P, P], fp16, tag="ident_h")
        nc.vector.tensor_copy(ident[:], ident_f[:])

        QT = big.tile([P, NPT, S], fp16, tag="QT")
        KT = big.tile([P, NPT, S], fp16, tag="KT")
        OT = big.tile([P, NPT, S], fp16, tag="OT")
        VA = big.tile([P, HPC, SKT, P], fp16, tag="VA")
        wo_sb = big.tile([P, NPT, D], fp16, tag="wo")

        # --- input DMA ---
        # weights on the gpsimd (SWDGE) queue in consumption order, bulk xT
        # on the sync (HWDGE) queue; wo last (not needed until outproj).
        w_sb = {}
        for name, w in (("k", wk), ("q", wq), ("v", wv)):
            t = big.tile([P, NKT, M], fp16, tag=f"w{name}")
            nc.gpsimd.dma_start(t[:], w.rearrange("(kt p) m -> p kt m", p=P))
            w_sb[name] = t

        xT_sb = big.tile([P, NKT, S], fp16, tag="xT")
        xT_r = xT.rearrange("(kt p) s -> p kt s", p=P)
        for c in range(NQC):
            for kh in range(4 if c == 0 else 2):
                n = NKT // (4 if c == 0 else 2)
                nc.sync.dma_start(
                    xT_sb[:, kh * n:(kh + 1) * n, c * QC:(c + 1) * QC],
                    xT_r[:, kh * n:(kh + 1) * n, c * QC:(c + 1) * QC])
        nc.sync.dma_start(wo_sb[:], wo.rearrange("(pt p) n -> p pt n", p=P))

        # ones block of VA for softmax row-sums
        ones_f = const.tile([P, 64], fp16, tag="ones")
        nc.gpsimd.memset(ones_f[:], 1.0)
        for h in range(HPC):
            for st in range(SKT):
                nc.gpsimd.tensor_copy(VA[:, h, st, 64:128], ones_f[:])

        # warm the PE clock (HAM) during the input DMA head
        for _ in range(24):
            warm_ps = psum_big.tile([P, QC], f32, tag="ps_big")
            nc.tensor.matmul(warm_ps[:, 0:P], ident[:], ident[:],
                             start=True, stop=True)

        def proj_group(name, pt, c):
            """One projection psum fill (8 accumulating matmuls + evac),
            split into 8 single-matmul side steps sharing lazy state."""
            state = {}
            dst = {"q": QT, "k": KT}[name]
            cs = slice(c * QC, (c + 1) * QC)

            def step(kt, state=state, cs=cs, dst=dst, name=name, pt=pt):
                if kt == 0:
                    state["ps"] = psum_big.tile([P, QC], f32, tag="ps_big",
                                                name="ps_proj")
                ps = state["ps"]
                nc.tensor.matmul(
                    ps[:], w_sb[name][:, kt, pt * P:(pt + 1) * P],
                    xT_sb[:, kt, cs],
                    start=(kt == 0), stop=(kt == NKT - 1))
                if kt == NKT - 1:
                    nc.vector.tensor_copy(dst[:, pt, cs], ps[:])

            return [lambda kt=kt: step(kt) for kt in range(NKT)]

        if "novdirect" in KV:
            f32r = mybir.dt.float32r
            ident_r = const.tile([P, P], f32r, tag="ident_r")
            nc.vector.tensor_copy(ident_r[:], ident_f[:])
            VT = big.tile([P, NPT, S], f32r, tag="VT")

            def v_group(st):
                # baseline path: transposed V proj (once, upfront via novjit)
                def thunk(st=st):
                    if st == 0:
                        for pt in range(NPT):
                            for c in range(NQC):
                                ps = psum_big.tile([P, QC], f32, tag="ps_big")
                                cs = slice(c * QC, (c + 1) * QC)
                                for kt in range(NKT):
                                    nc.tensor.matmul(
                                        ps[:],
                                        w_sb["v"][:, kt, pt * P:(pt + 1) * P],
                                        xT_sb[:, kt, cs],
                                        start=(kt == 0), stop=(kt == NKT - 1))
                                nc.vector.tensor_copy(VT[:, pt, cs], ps[:])
                    for pt in range(NPT):
                        pst = psum_big.tile([P, P], f32r, tag="ps_big")
                        nc.tensor.transpose(
                            pst[:], VT[:, pt, st * P:(st + 1) * P], ident_r)
                        nc.vector.tensor_copy(VA[:, 2 * pt, st, 0:64],
                                              pst[:, 0:64])
                        nc.vector.tensor_copy(VA[:, 2 * pt + 1, st, 0:64],
                                              pst[:, 64:128])
                return thunk
        else:
            def v_group(st):
                """Project V for seq-tile st directly into natural layout and
                scatter per-head slices (+CAST) into VA. One side-thunk."""
                def thunk(st=st):
                    ps = psum_big.tile([P, QC], f32, tag="ps_big", name="ps_v")
                    ss = slice(st * P, (st + 1) * P)
                    for kt in range(NKT):
                        nc.tensor.matmul(
                            ps[:, 0:M], xT_sb[:, kt, ss], w_sb["v"][:, kt, :],
                            start=(kt == 0), stop=(kt == NKT - 1))
                    for h in range(HPC):
                        nc.vector.tensor_copy(VA[:, h, st, 0:64],
                                              ps[:, h * 64:(h + 1) * 64])
                return thunk

        def outproj_piece(qt, nch):
            """Two single-matmul steps; the second evacuates + DMAs."""
            state = {}

            def step(pt, qt=qt, nch=nch, state=state):
                if pt == 0:
                    state["ps"] = psum_big.tile([P, QC], f32, tag="ps_big",
                                                name="ps_op")
                ps = state["ps"]
                nc.tensor.matmul(
                    ps[:], OT[:, pt, qt * P:(qt + 1) * P],
                    wo_sb[:, pt, nch * QC:(nch + 1) * QC],
                    start=(pt == 0), stop=(pt == NPT - 1))
                if pt == NPT - 1:
                    ot = work.tile([P, QC], f32, tag="outstage")
                    nc.vector.tensor_copy(ot[:], ps[:])
                    nc.sync.dma_start(
                        out[qt * P:(qt + 1) * P, nch * QC:(nch + 1) * QC],
                        ot[:])

            return [lambda pt=pt: step(pt) for pt in range(NPT)]

        # --- K, Q (pair 0, first q-chunk, first key window) so scores can
        # start as soon as the first xT chunk lands; sc(0,k2) only reads
        # KT key-tile k2, so the remaining K chunks project as early
        # side-work just ahead of their first consumer. ---
        for step in proj_group("k", 0, 0):
            step()
        for step in proj_group("q", 0, 0):
            step()

        if "noside" in KV:
            # bisect variant: all projections upfront, no interleave
            for c in range(1, NQC):
                for step in proj_group("q", 0, c):
                    step()
            for name in ("k", "q"):
                for c in range(NQC):
                    for step in proj_group(name, 1, c):
                        step()

        # --- attention pipeline ---
        # chunks in pair-major order; AV lags the exp stream by one chunk
        # (et pool is deep), side-work interleaved per k2 unit.
        chunks = [(p, c) for p in range(NPT) for c in range(NQC)]

        # side-work lists per chunk (index 9 = tail), scripted so every
        # projection lands >= one chunk before its first consumer; the
        # light outproj chunks (4-7) absorb pair-1 Q so the exp-critical
        # early chunks stay close to the ACT cadence.
        side = {i: [] for i in range(10)}
        if "noside" not in KV:
            side[0] += proj_group("k", 0, 1) + proj_group("k", 0, 2) \
                + proj_group("k", 0, 3) + proj_group("q", 0, 1)
            side[1] += proj_group("q", 0, 2) + proj_group("k", 1, 0)
            side[2] += proj_group("q", 0, 3) + proj_group("k", 1, 1)
            side[3] += proj_group("k", 1, 2) + proj_group("k", 1, 3) \
                + proj_group("q", 1, 0)
            side[4] += proj_group("q", 1, 1)
            side[5] += proj_group("q", 1, 2)
            side[6] += proj_group("q", 1, 3)
        # outproj thunks for window c are appended to side[...] by norm()

        vq = [v_group(st) for st in range(SKT)]
        if "novjit" in KV:
            for th in vq:
                th()
            vq = [(lambda: None) for _ in range(SKT)]
        ets = {}     # global unit -> et tile
        pos = {}     # chunk_idx -> (poA, poB)

        def sc_unit(u):
            i, k2 = divmod(u, SKT)
            p, c = chunks[i]
            cs = slice(c * QC, (c + 1) * QC)
            ks = slice(k2 * P, (k2 + 1) * P)
            pss = psum_s.tile([P, 2, QC], f32, tag="ps_s")
            for j in range(2):
                nc.tensor.matmul(pss[:, j, :], KT[64 * j:64 * (j + 1), p, ks],
                                 QT[64 * j:64 * (j + 1), p, cs],
                                 start=True, stop=True)
            et = exp_pool.tile([P, 2, QC], fp16, tag="exp")
            nc.scalar.activation(et[:], pss[:], AF.Exp, scale=SCALE)
            ets[u] = et

        def av_unit(u):
            i, k2 = divmod(u, SKT)
            p, c = chunks[i]
            et = ets.pop(u)
            if k2 == 0:
                poA = psum_o.tile([P, QC], f32, tag="ps_oA")
                poB = psum_o.tile([P, QC], f32, tag="ps_oB")
                pos[i] = (poA, poB)
            poA, poB = pos[i]
            for j, po in ((0, poA), (1, poB)):
                nc.tensor.matmul(po[:], VA[:, 2 * p + j, k2, :], et[:, j, :],
                                 start=(k2 == 0), stop=(k2 == SKT - 1))
            if k2 == SKT - 1:
                norm(i)

        def norm(i):
            p, c = chunks[i]
            cs = slice(c * QC, (c + 1) * QC)
            for r0, po in ((0, pos[i][0]), (64, pos[i][1])):
                pcv = small.tile([64, QC], f32, tag="po_val")
                nc.vector.tensor_copy(pcv[:], po[0:64, :])
                pcs = small.tile([64, QC], f32, tag="po_sum")
                nc.vector.tensor_copy(pcs[:], po[64:128, :])
                rb = small.tile([64, QC], f32, tag="recip")
                nc.vector.reciprocal_approx_fast(rb[:], pcs[:])
                nc.vector.tensor_tensor(
                    OT[r0:r0 + 64, p, cs], pcv[:], rb[:],
                    mybir.AluOpType.mult)
            del pos[i]
            if p == 1:
                # OT complete for this q window: queue its outproj
                tgt = 9 if ("noopi" in KV or i == 7) else i + 1
                for qt in range(c * (QC // P), (c + 1) * (QC // P)):
                    for nch in range(2):
                        side[tgt].extend(outproj_piece(qt, nch))

        # --- global unit pipeline: av (lagging LAG units, emitted first so
        # its wait absorbs the exp latency), then side-work, then sc ---
        LAG = 2
        NU = 8 * SKT
        for u in range(NU):
            i, k2 = divmod(u, SKT)
            if u >= LAG:
                av_unit(u - LAG)
            if i == 0:
                vq[k2]()
            sq = side[i]
            for _ in range(min(2, len(sq))):
                sq.pop(0)()
            if k2 == SKT - 1:
                while sq:
                    sq.pop(0)()
            sc_unit(u)
        for u in range(NU - LAG, NU):
            av_unit(u)
        # keep the PE clock hot through the final norm so the tail outproj
        # runs warm
        for _ in range(10):
            wp = psum_big.tile([P, QC], f32, tag="ps_big", name="tailwarm")
            nc.tensor.matmul(wp[:, 0:P], ident[:], ident[:],
                             start=True, stop=True)
        for th in side[8] + side[9]:
            th()
        assert not ets and not pos


def _in_maps(x, Wq, Wk, Wv, Wo):
    x = np.asarray(x, dtype=np.float32)
    Wq = np.asarray(Wq, dtype=np.float32)
    Wk = np.asarray(Wk, dtype=np.float32)
    Wv = np.asarray(Wv, dtype=np.float32)
    Wo = np.asarray(Wo, dtype=np.float32)
    xT = [np.ascontiguousarray(x[b].T).astype(np.float16) for b in range(B)]
    maps = []
    for c in range(NCORES):
        b, g = c // GROUPS, c % GROUPS
        rows = slice(g * M, (g + 1) * M)
        maps.append({
            "xT": xT[b],
            "wq": np.ascontiguousarray(Wq[rows, :].T).astype(np.float16),
            "wk": np.ascontiguousarray(Wk[rows, :].T).astype(np.float16),
            "wv": np.ascontiguousarray(Wv[rows, :].T).astype(np.float16),
            "wo": np.ascontiguousarray(Wo[:, rows].T).astype(np.float16),
        })
    return maps


def kernel(x, Wq, Wk, Wv, Wo, _trace=False):
    global _compiled
    if _compiled is None:
        _compiled = _build_module()
    from concourse.bass_utils import run_bass_kernel_spmd

    res = run_bass_kernel_spmd(
        _compiled, _in_maps(x, Wq, Wk, Wv, Wo),
        core_ids=list(range(NCORES)), trace=_trace,
    )
    outs = [r["out"] for r in res.results]
    y = np.empty((B, S, D), np.float32)
    for b in range(B):
        y[b] = outs[4 * b] + outs[4 * b + 1] + outs[4 * b + 2] + outs[4 * b + 3]
    if _trace:
        kernel.last_results = res
    return y


# revision 30
# speedup vs baseline: 1.0423x; 1.0277x over previous
"""Multi-head attention (B=2, S=2048, D=1024, H=16) on 8 TRN2 NeuronCores.

Sharding: tensor-parallel over heads x data-parallel over batch.
Core c handles batch b = c // 4 and head group g = c % 4 (4 heads each).
Each core computes its 4 heads' q/k/v projections, attention, and the
partial output projection against its slice of Wo; the host sums the 4
partials per batch element.

Schedule: the attention phase is paced by the ACT engine's exp stream
(128 exp tiles of [128,1024] at ~1.1us each), with the PE near-saturated
around it, so the kernel is organized as a single software-pipelined
stream of 128 "units" (8 chunks of (head-pair, q-window) x 16 key
tiles):
  - Only K and Q for (pair 0, q-window 0) project up front; sc(0,k2)
    reads just key-tile k2, so scoring starts as soon as the first xT
    DMA chunk lands. All remaining projections are split into
    single-matmul side-steps popped between units, each scheduled at
    least one chunk before its first consumer.
  - Unit u: AV for unit u-2 (its exp long done - the wait absorbs the
    exp->pss round-trip), side-steps, then scores for unit u; the two
    per-pair score matmuls contract Dh=64 on partition ranges
    0:64/64:128 so their tile_position row groups (0,0)/(64,0) run
    concurrently on the PE array.
  - V is projected directly into natural [seq, feat] layout (stationary
    = xT seq-tile, moving = Wv; no PE transposes), just-in-time per key
    tile inside chunk 0.
  - The output projection is split into single-matmul side-steps that
    follow each q-window's completion, spreading the output DMA across
    the kernel; ident matmuls keep the PE clock (HAM) warm through the
    final normalization so the tail outproj runs at full clock.

fp16 streaming everywhere; accumulation in f32 PSUM. The AV matmul's
stationary carries 64 columns of ones so softmax row-sums come out of
the same matmul (partition-broadcast is illegal on DVE; replicating in
the matmul is free). Normalization (reciprocal * mult) on DVE via two
aligned 64-row evacuations; note reciprocal_approx_fast requires a
base-partition-0 input on real hardware (a partition-shifted input
returns garbage even though CoreSim accepts it). ACT runs exp only.
"""

import os

import numpy as np

# bisection switches (dev only; default full-featured)
KV = set(os.environ.get("KV", "").split(","))

B, S, D, H, DH = 2, 2048, 1024, 16, 64
NCORES = 8
GROUPS = 4  # head groups; 4 heads = 256 features per core
M = 256  # head features per core
SCALE = 0.125  # 1/sqrt(64)

_compiled = None


def _build_module():
    import concourse.mybir as mybir
    import concourse.tile as tile
    from concourse import bacc

    f32 = mybir.dt.float32
    fp16 = mybir.dt.float16
    nc = bacc.Bacc("TRN2", target_bir_lowering=False, debug=False,
                   num_devices=NCORES)
    xT = nc.dram_tensor("xT", [D, S], fp16, kind="ExternalInput").ap()
    wq = nc.dram_tensor("wq", [D, M], fp16, kind="ExternalInput").ap()
    wk = nc.dram_tensor("wk", [D, M], fp16, kind="ExternalInput").ap()
    wv = nc.dram_tensor("wv", [D, M], fp16, kind="ExternalInput").ap()
    wo = nc.dram_tensor("wo", [M, D], fp16, kind="ExternalInput").ap()
    out = nc.dram_tensor("out", [S, D], f32, kind="ExternalOutput").ap()

    with tile.TileContext(nc) as tc:
        _kernel_body(tc, out, xT, wq, wk, wv, wo)
    nc.compile()
    return nc


def _kernel_body(tc, out, xT, wq, wk, wv, wo):
    from contextlib import ExitStack

    import concourse.mybir as mybir
    from concourse.masks import make_identity

    nc = tc.nc
    f32 = mybir.dt.float32
    fp16 = mybir.dt.float16
    AF = mybir.ActivationFunctionType

    P = 128
    NKT = D // P   # 8 k-tiles in the projection contraction
    NPT = M // P   # 2 partition-tiles of head features (head pairs)
    SKT = S // P   # 16 key tiles
    QC = 512       # q chunk (psum bank width in f32)
    NQC = S // QC  # 4
    HPC = 4        # heads per core

    with ExitStack() as ctx:
        const = ctx.enter_context(tc.tile_pool(name="const", bufs=1))
        big = ctx.enter_context(tc.tile_pool(name="big", bufs=1))
        work = ctx.enter_context(tc.tile_pool(name="work", bufs=2))
        exp_pool = ctx.enter_context(
            tc.tile_pool(name="exp", bufs=20 if "lag" in KV else 8))
        small = ctx.enter_context(tc.tile_pool(name="small", bufs=2))
        # PSUM budget (8 banks): psA 2 + psS 2x2 + psO 2x1 = 8
        psum_big = ctx.enter_context(tc.tile_pool(name="psA", bufs=2, space="PSUM"))
        psum_s = ctx.enter_context(tc.tile_pool(name="psS", bufs=2, space="PSUM"))
        psum_o = ctx.enter_context(tc.tile_pool(name="psO", bufs=1, space="PSUM"))

        ident_f = const.tile([P, P], f32)
        make_identity(nc, ident_f)
        ident = const.tile([P, P], fp16, tag="ident_h")
        nc.vector.tensor_copy(ident[:], ident_f[:])

        QT = big.tile([P, NPT, S], fp16, tag="QT")
        KT = big.tile([P, NPT, S], fp16, tag="KT")
        OT = big.tile([P, NPT, S], fp16, tag="OT")
        VA = big.tile([P, HPC, SKT, P], fp16, tag="VA")
        wo_sb = big.tile([P, NPT, D], fp16, tag="wo")

        # --- input DMA ---
        # weights on the gpsimd (SWDGE) queue in consumption order, bulk xT
        # on the sync (HWDGE) queue; wo last (not needed until outproj).
        w_sb = {}
        for name, w in (("k", wk), ("q", wq), ("v", wv)):
            t = big.tile([P, NKT, M], fp16, tag=f"w{name}")
            nc.gpsimd.dma_start(t[:], w.rearrange("(kt p) m -> p kt m", p=P))
            w_sb[name] = t

        xT_sb = big.tile([P, NKT, S], fp16, tag="xT")
        xT_r = xT.rearrange("(kt p) s -> p kt s", p=P)
        for c in range(NQC):
            for kh in range(4 if c == 0 else 2):
                n = NKT // (4 if c == 0 else 2)
                nc.sync.dma_start(
                    xT_sb[:, kh * n:(kh + 1) * n, c * QC:(c + 1) * QC],
                    xT_r[:, kh * n:(kh + 1) * n, c * QC:(c + 1) * QC])
        nc.sync.dma_start(wo_sb[:], wo.rearrange("(pt p) n -> p pt n", p=P))

        # ones block of VA for softmax row-sums
        ones_f = const.tile([P, 64], fp16, tag="ones")
        nc.gpsimd.memset(ones_f[:], 1.0)
        for h in range(HPC):
            for st in range(SKT):
                nc.gpsimd.tensor_copy(VA[:, h, st, 64:128], ones_f[:])

        # warm the PE clock (HAM) during the input DMA head
        for _ in range(24):
            warm_ps = psum_big.tile([P, QC], f32, tag="ps_big")
            nc.tensor.matmul(warm_ps[:, 0:P], ident[:], ident[:],
                             start=True, stop=True)

        def proj_group(name, pt, c):
            """One projection psum fill (8 accumulating matmuls + evac),
            split into 8 single-matmul side steps sharing lazy state."""
            state = {}
            dst = {"q": QT, "k": KT}[name]
            cs = slice(c * QC, (c + 1) * QC)

            def step(kt, state=state, cs=cs, dst=dst, name=name, pt=pt):
                if kt == 0:
                    state["ps"] = psum_big.tile([P, QC], f32, tag="ps_big",
                                                name="ps_proj")
                ps = state["ps"]
                nc.tensor.matmul(
                    ps[:], w_sb[name][:, kt, pt * P:(pt + 1) * P],
                    xT_sb[:, kt, cs],
                    start=(kt == 0), stop=(kt == NKT - 1))
                if kt == NKT - 1:
                    nc.vector.tensor_copy(dst[:, pt, cs], ps[:])

            return [lambda kt=kt: step(kt) for kt in range(NKT)]

        if "novdirect" in KV:
            f32r = mybir.dt.float32r
            ident_r = const.tile([P, P], f32r, tag="ident_r")
            nc.vector.tensor_copy(ident_r[:], ident_f[:])
            VT = big.tile([P, NPT, S], f32r, tag="VT")

            def v_group(st):
                # baseline path: transposed V proj (once, upfront via novjit)
                def thunk(st=st):
                    if st == 0:
                        for pt in range(NPT):
                            for c in range(NQC):
                                ps = psum_big.tile([P, QC], f32, tag="ps_big")
                                cs = slice(c * QC, (c + 1) * QC)
                                for kt in range(NKT):
                                    nc.tensor.matmul(
                                        ps[:],
                                        w_sb["v"][:, kt, pt * P:(pt + 1) * P],
                                        xT_sb[:, kt, cs],
                                        start=(kt == 0), stop=(kt == NKT - 1))
                                nc.vector.tensor_copy(VT[:, pt, cs], ps[:])
                    for pt in range(NPT):
                        pst = psum_big.tile([P, P], f32r, tag="ps_big")
                        nc.tensor.transpose(
                            pst[:], VT[:, pt, st * P:(st + 1) * P], ident_r)
                        nc.vector.tensor_copy(VA[:, 2 * pt, st, 0:64],
                                              pst[:, 0:64])
                        nc.vector.tensor_copy(VA[:, 2 * pt + 1, st, 0:64],
                                              pst[:, 64:128])
                return thunk
        else:
            def v_group(st):
                """Project V for seq-tile st directly into natural layout and
                scatter per-head slices (+CAST) into VA. One side-thunk."""
                def thunk(st=st):
                    ps = psum_big.tile([P, QC], f32, tag="ps_big", name="ps_v")
                    ss = slice(st * P, (st + 1) * P)
                    for kt in range(NKT):
                        nc.tensor.matmul(
                            ps[:, 0:M], xT_sb[:, kt, ss], w_sb["v"][:, kt, :],
                            start=(kt == 0), stop=(kt == NKT - 1))
                    for h in range(HPC):
                        nc.vector.tensor_copy(VA[:, h, st, 0:64],
                                              ps[:, h * 64:(h + 1) * 64])
                return thunk

        def outproj_piece(qt, nch):
            """Two single-matmul steps; the second evacuates + DMAs."""
            state = {}

            def step(pt, qt=qt, nch=nch, state=state):
                if pt == 0:
                    state["ps"] = psum_big.tile([P, QC], f32, tag="ps_big",
                                                name="ps_op")
                ps = state["ps"]
                nc.tensor.matmul(
                    ps[:], OT[:, pt, qt * P:(qt + 1) * P],
                    wo_sb[:, pt, nch * QC:(nch + 1) * QC],
                    start=(pt == 0), stop=(pt == NPT - 1))
                if pt == NPT - 1:
                    ot = work.tile([P, QC], f32, tag="outstage")
                    nc.vector.tensor_copy(ot[:], ps[:])
                    nc.sync.dma_start(
                        out[qt * P:(qt + 1) * P, nch * QC:(nch + 1) * QC],
                        ot[:])

            return [lambda pt=pt: step(pt) for pt in range(NPT)]

        # --- K, Q (pair 0, first q-chunk, first key window) so scores can
        # start as soon as the first xT chunk lands; sc(0,k2) only reads
        # KT key-tile k2, so the remaining K chunks project as early
        # side-work just ahead of their first consumer. ---
        for step in proj_group("k", 0, 0):
            step()
        for step in proj_group("q", 0, 0):
            step()

        if "noside" in KV:
            # bisect variant: all projections upfront, no interleave
            for c in range(1, NQC):
                for step in proj_group("q", 0, c):
                    step()
            for name in ("k", "q"):
                for c in range(NQC):
                    for step in proj_group(name, 1, c):
                        step()

        # --- attention pipeline ---
        # chunks in pair-major order; AV lags the exp stream by one chunk
        # (et pool is deep), side-work interleaved per k2 unit.
        chunks = [(p, c) for p in range(NPT) for c in range(NQC)]

        # side-work lists per chunk (index 9 = tail), scripted so every
        # projection lands >= one chunk before its first consumer; the
        # light outproj chunks (4-7) absorb pair-1 Q so the exp-critical
        # early chunks stay close to the ACT cadence.
        side = {i: [] for i in range(10)}
        if "noside" not in KV:
            side[0] += proj_group("k", 0, 1) + proj_group("k", 0, 2) \
                + proj_group("k", 0, 3) + proj_group("q", 0, 1)
            side[1] += proj_group("q", 0, 2) + proj_group("k", 1, 0)
            side[2] += proj_group("q", 0, 3) + proj_group("k", 1, 1)
            side[3] += proj_group("k", 1, 2) + proj_group("k", 1, 3) \
                + proj_group("q", 1, 0)
            side[4] += proj_group("q", 1, 1)
            side[5] += proj_group("q", 1, 2)
            side[6] += proj_group("q", 1, 3)
        # outproj thunks for window c are appended to side[...] by norm()

        vq = [v_group(st) for st in range(SKT)]
        if "novjit" in KV:
            for th in vq:
                th()
            vq = [(lambda: None) for _ in range(SKT)]
        ets = {}     # global unit -> et tile
        pos = {}     # chunk_idx -> (poA, poB)

        def sc_unit(u):
            i, k2 = divmod(u, SKT)
            p, c = chunks[i]
            cs = slice(c * QC, (c + 1) * QC)
            ks = slice(k2 * P, (k2 + 1) * P)
            pss = psum_s.tile([P, 2, QC], f32, tag="ps_s")
            for j in range(2):
                nc.tensor.matmul(pss[:, j, :], KT[64 * j:64 * (j + 1), p, ks],
                                 QT[64 * j:64 * (j + 1), p, cs],
                                 start=True, stop=True)
            et = exp_pool.tile([P, 2, QC], fp16, tag="exp")
            nc.scalar.activation(et[:], pss[:], AF.Exp, scale=SCALE)
            ets[u] = et

        def av_unit(u):
            i, k2 = divmod(u, SKT)
            p, c = chunks[i]
            et = ets.pop(u)
            if k2 == 0:
                poA = psum_o.tile([P, QC], f32, tag="ps_oA")
                poB = psum_o.tile([P, QC], f32, tag="ps_oB")
                pos[i] = (poA, poB)
            poA, poB = pos[i]
            for j, po in ((0, poA), (1, poB)):
                nc.tensor.matmul(po[:], VA[:, 2 * p + j, k2, :], et[:, j, :],
                                 start=(k2 == 0), stop=(k2 == SKT - 1))
            if k2 == SKT - 1:
                norm(i)

        def norm(i):
            p, c = chunks[i]
            cs = slice(c * QC, (c + 1) * QC)
            for r0, po in ((0, pos[i][0]), (64, pos[i][1])):
                pcv = small.tile([64, QC], f32, tag="po_val")
                nc.vector.tensor_copy(pcv[:], po[0:64, :])
                pcs = small.tile([64, QC], f32, tag="po_sum")
                nc.vector.tensor_copy(pcs[:], po[64:128, :])
                rb = small.tile([64, QC], f32, tag="recip")
                nc.vector.reciprocal_approx_fast(rb[:], pcs[:])
                nc.vector.tensor_tensor(
                    OT[r0:r0 + 64, p, cs], pcv[:], rb[:],
                    mybir.AluOpType.mult)
            del pos[i]
            if p == 1:
                # OT complete for this q window: queue its outproj
                tgt = 9 if ("noopi" in KV or i == 7) else i + 1
                for qt in range(c * (QC // P), (c + 1) * (QC // P)):
                    for nch in range(2):
                        side[tgt].extend(outproj_piece(qt, nch))

        # --- global unit pipeline: av (lagging LAG units, emitted first so
        # its wait absorbs the exp latency), then side-work, then sc ---
        LAG = 4
        NU = 8 * SKT
        for u in range(NU):
            i, k2 = divmod(u, SKT)
            if u >= LAG:
                av_unit(u - LAG)
            if i == 0:
                vq[k2]()
            sq = side[i]
            for _ in range(min(2, len(sq))):
                sq.pop(0)()
            if k2 == SKT - 1:
                while sq:
                    sq.pop(0)()
            sc_unit(u)
        for u in range(NU - LAG, NU):
            av_unit(u)
        # keep the PE clock hot through the final norm so the tail outproj
        # runs warm
        for _ in range(10):
            wp = psum_big.tile([P, QC], f32, tag="ps_big", name="tailwarm")
            nc.tensor.matmul(wp[:, 0:P], ident[:], ident[:],
                             start=True, stop=True)
        for th in side[8] + side[9]:
            th()
        assert not ets and not pos


def _in_maps(x, Wq, Wk, Wv, Wo):
    x = np.asarray(x, dtype=np.float32)
    Wq = np.asarray(Wq, dtype=np.float32)
    Wk = np.asarray(Wk, dtype=np.float32)
    Wv = np.asarray(Wv, dtype=np.float32)
    Wo = np.asarray(Wo, dtype=np.float32)
    xT = [np.ascontiguousarray(x[b].T).astype(np.float16) for b in range(B)]
    maps = []
    for c in range(NCORES):
        b, g = c // GROUPS, c % GROUPS
        rows = slice(g * M, (g + 1) * M)
        maps.append({
            "xT": xT[b],
            "wq": np.ascontiguousarray(Wq[rows, :].T).astype(np.float16),
            "wk": np.ascontiguousarray(Wk[rows, :].T).astype(np.float16),
            "wv": np.ascontiguousarray(Wv[rows, :].T).astype(np.float16),
            "wo": np.ascontiguousarray(Wo[:, rows].T).astype(np.float16),
        })
    return maps


def kernel(x, Wq, Wk, Wv, Wo, _trace=False):
    global _compiled
    if _compiled is None:
        _compiled = _build_module()
    from concourse.bass_utils import run_bass_kernel_spmd

    res = run_bass_kernel_spmd(
        _compiled, _in_maps(x, Wq, Wk, Wv, Wo),
        core_ids=list(range(NCORES)), trace=_trace,
    )
    outs = [r["out"] for r in res.results]
    y = np.empty((B, S, D), np.float32)
    for b in range(B):
        y[b] = outs[4 * b] + outs[4 * b + 1] + outs[4 * b + 2] + outs[4 * b + 3]
    if _trace:
        kernel.last_results = res
    return y
